# revision 1
# baseline (speedup 1.0000x reference)
"""Trainium2 Bass kernel for a BasicTransformerBlock (self-attn + cross-attn + GEGLU FF).

Sharding: data-parallel over the batch axis — 8 batch elements onto 8 NeuronCores,
same SPMD program, no collectives.

Per-core design: the residual stream is kept TRANSPOSED in SBUF as xT[d, s]
(features on partitions, 4 partition-tiles x 1024 tokens), so every linear layer
runs directly off the HBM weight layout with no activation transposes:
    yT = W.T @ hT      (q/k projections, out-projs, FF; lhsT = W)
    v  = hT.T @ Wv     (attention values, tokens on partitions; lhsT = hT)
LayerNorm stats are cross-partition sums done on the PE with a ones lhsT; the
per-token mean/rstd rows are broadcast back across partitions with K=1 ones
matmuls. Softmax skips max-subtraction (scores are O(1) for this problem), exp
runs on ACT straight out of PSUM, and the softmax denominator is fused into the
probs@v matmul by appending a ones column to v (denominator lands in psum row 64).
All big matmuls run in float32r (1 cycle/row vs 4 for fp32, ~1.5e-4 rel err).
"""
import sys

sys.path.insert(0, "/opt/trn_rl_repo")

from contextlib import ExitStack

import numpy as np

import concourse.bass as bass
import concourse.mybir as mybir
import concourse.tile as tile
from concourse import bacc
from concourse.bass_utils import run_bass_kernel_spmd
from concourse.masks import make_identity

F32 = mybir.dt.float32
F32R = mybir.dt.float32r
AF = mybir.ActivationFunctionType
ALU = mybir.AluOpType

B = 8
S = 1024          # tokens
D = 512           # model dim
SK2 = 77          # cross-attention source length
DE = 768          # encoder dim
FF = 2048         # GEGLU inner dim (per half)
NH = 8            # heads
DH = 64           # head dim
SCALE = DH ** -0.5
EPS = 1e-5
P = 128
NC_ = 512         # token chunk (one psum bank of fp32)
ST = S // P       # 8 token tiles
FT = D // P       # 4 feature tiles
CH = S // NC_     # 2 token chunks
KE = DE // P      # 6 encoder feature tiles
NI = FF // P      # 16 FF inner tiles


def build(nc: bass.Bass):
    x = nc.dram_tensor("x", [S, D], F32, kind="ExternalInput")
    enc = nc.dram_tensor("enc", [SK2, DE], F32, kind="ExternalInput")
    w_in = {}
    for name, shape in [
        ("wq1", [D, D]), ("wk1", [D, D]), ("wv1", [D, D]), ("wo1", [D, D]),
        ("wq2", [D, D]), ("wk2", [DE, D]), ("wv2", [DE, D]), ("wo2", [D, D]),
        ("wg", [D, 2 * FF]), ("wf", [FF, D]),
    ]:
        w_in[name] = nc.dram_tensor(name, shape, F32, kind="ExternalInput")
    vec_in = {}
    for name, n in [("ln1_g", D), ("ln1_b", D), ("ln2_g", D), ("ln2_b", D),
                    ("ln3_g", D), ("ln3_b", D), ("bo1", D), ("bo2", D),
                    ("bg", 2 * FF), ("bf", D)]:
        vec_in[name] = nc.dram_tensor(name, [n], F32, kind="ExternalInput")
    out = nc.dram_tensor("out", [S, D], F32, kind="ExternalOutput")

    with tile.TileContext(nc) as tc, ExitStack() as es:
        const = es.enter_context(tc.tile_pool(name="const", bufs=1))
        resid = es.enter_context(tc.tile_pool(name="resid", bufs=2))
        keep = es.enter_context(tc.tile_pool(name="keep", bufs=1))
        stage = es.enter_context(tc.tile_pool(name="stage", bufs=2))
        tmp = es.enter_context(tc.tile_pool(name="tmp", bufs=2))
        rowp = es.enter_context(tc.tile_pool(name="rowp", bufs=2))
        scrp = es.enter_context(tc.tile_pool(name="scrp", bufs=2))
        ex_pool = es.enter_context(tc.tile_pool(name="ex", bufs=6))
        bcp = es.enter_context(tc.tile_pool(name="bcp", bufs=3))
        ps_t = es.enter_context(tc.tile_pool(name="ps_t", bufs=4, space="PSUM"))
        ps_a = es.enter_context(tc.tile_pool(name="ps_a", bufs=4, space="PSUM"))

        # ---- constants ----
        ident = const.tile([P, P], F32)
        make_identity(nc, ident[:])
        ones_f = const.tile([P, P], F32)
        nc.vector.memset(ones_f[:], 1.0)
        ones128 = const.tile([P, 1], F32R)          # stats lhsT (K=128, M=1)
        nc.vector.tensor_copy(ones128[:], ones_f[:, 0:1])
        ones1x = const.tile([1, P], F32R)           # bcast lhsT (K=1, M<=128)
        nc.vector.tensor_copy(ones1x[:], ones_f[0:1, :])
        eps_t = const.tile([1, 1], F32)
        nc.vector.memset(eps_t[:], EPS)

        def col_const(name, n):
            t = const.tile([P, n], F32, tag=f"{name}_c")
            nc.sync.dma_start(t[:], vec_in[name].rearrange("(o p) -> p o", p=P))
            return t

        g1c, b1c = col_const("ln1_g", FT), col_const("ln1_b", FT)
        g2c, b2c = col_const("ln2_g", FT), col_const("ln2_b", FT)
        g3c, b3c = col_const("ln3_g", FT), col_const("ln3_b", FT)
        bo1c, bo2c = col_const("bo1", FT), col_const("bo2", FT)
        bfc = col_const("bf", FT)
        bgc = col_const("bg", 2 * NI)   # [:, 0:16]=u biases, [:, 16:32]=g biases

        def load_w(pool, name, kouter, tag):
            """Stream a [K, N<=512] HBM weight into [128, kouter, N] f32r."""
            dram = w_in[name]
            nout = dram.shape[1]
            wr = pool.tile([P, kouter, nout], F32R, tag=tag)
            for k in range(kouter):
                stg = stage.tile([P, nout], F32, tag="wst")
                nc.sync.dma_start(stg[:], dram[P * k:P * (k + 1), :])
                nc.gpsimd.tensor_copy(wr[:, k, :], stg[:])
            return wr

        # ---- load x, transpose into xT [128, FT, S] (f32r) ----
        xT = resid.tile([P, FT, S], F32R, tag="x")
        for st in range(ST):
            xr = stage.tile([P, D], F32, tag="x_raw")
            nc.sync.dma_start(xr[:], x[P * st:P * (st + 1), :])
            pt = ps_t.tile([P, NC_], F32, tag="t")
            for ft in range(FT):
                nc.tensor.transpose(pt[:, P * ft:P * (ft + 1)],
                                    xr[:, P * ft:P * (ft + 1)], ident[:])
            nc.vector.tensor_copy(
                xT[:, :, P * st:P * (st + 1)],
                pt[:].rearrange("p (f q) -> p f q", f=FT))

        # ---- load enc, transpose into encT [128, KE, 78] (f32r; col 77 zero-pad
        # because f32r matmuls need an even moving-dim) ----
        SK2P = SK2 + 1
        zeros_f = const.tile([P, 1], F32)
        nc.vector.memset(zeros_f[:], 0.0)
        encT = keep.tile([P, KE, SK2P], F32R, tag="encT")
        with tc.tile_pool(name="encstg", bufs=1) as encstg:
            enc_raw = encstg.tile([SK2, DE], F32, tag="enc_raw")
            nc.sync.dma_start(enc_raw[:], enc[:, :])
            for ke in range(KE):
                pt = ps_t.tile([P, NC_], F32, tag="t")
                nc.tensor.transpose(pt[:, 0:SK2], enc_raw[:, P * ke:P * (ke + 1)],
                                    ident[0:SK2, 0:SK2])
                nc.vector.tensor_copy(encT[:, ke, 0:SK2], pt[:, 0:SK2])
                nc.vector.tensor_copy(encT[:, ke, SK2:SK2P], zeros_f[:, 0:1])

        def layer_norm(src, gcol, bcol, pool, tag):
            """src [128, FT, S] f32r -> hT [128, FT, S] f32r."""
            hT = pool.tile([P, FT, S], F32R, tag=tag)
            for c in range(CH):
                cs = slice(NC_ * c, NC_ * (c + 1))
                sum_ps = ps_a.tile([1, NC_], F32, tag="a")
                for ft in range(FT):
                    nc.tensor.matmul(sum_ps[:], ones128[:], src[:, ft, cs],
                                     start=(ft == 0), stop=(ft == FT - 1))
                sq_ps = ps_a.tile([1, NC_], F32, tag="a")
                for ft in range(FT):
                    xsq = tmp.tile([P, NC_], F32R, tag="xsq")
                    nc.scalar.activation(xsq[:], src[:, ft, cs], AF.Square)
                    nc.tensor.matmul(sq_ps[:], ones128[:], xsq[:],
                                     start=(ft == 0), stop=(ft == FT - 1))
                mu = rowp.tile([1, NC_], F32, tag="mu")
                nc.vector.tensor_scalar_mul(mu[:], sum_ps[:], 1.0 / D)
                musq = scrp.tile([1, NC_], F32, tag="scr")
                nc.vector.tensor_mul(musq[:], mu[:], mu[:])
                var = scrp.tile([1, NC_], F32, tag="scr")
                nc.vector.scalar_tensor_tensor(
                    var[:], sq_ps[:], 1.0 / D, musq[:], op0=ALU.mult, op1=ALU.subtract)
                sd = scrp.tile([1, NC_], F32, tag="scr")
                nc.scalar.activation(sd[:], var[:], AF.Sqrt, bias=eps_t[:])
                rstd = rowp.tile([1, NC_], F32, tag="rstd")
                nc.vector.reciprocal(rstd[:], sd[:])
                mu_b = bcp.tile([P, NC_], F32, tag="mub")
                nc.gpsimd.partition_broadcast(mu_b[:], mu[:])
                rstd_b = bcp.tile([P, NC_], F32, tag="rstdb")
                nc.gpsimd.partition_broadcast(rstd_b[:], rstd[:])
                for ft in range(FT):
                    t = tmp.tile([P, NC_], F32, tag="lt")
                    nc.vector.tensor_tensor(t[:], src[:, ft, cs], mu_b[:], ALU.subtract)
                    nc.vector.tensor_tensor(t[:], t[:], rstd_b[:], ALU.mult)
                    nc.vector.tensor_scalar(
                        hT[:, ft, cs], t[:], gcol[:, ft:ft + 1], bcol[:, ft:ft + 1],
                        op0=ALU.mult, op1=ALU.add)
            return hT

        def project_T(w_r, src, kouter, pool, tag):
            """yT = W.T @ src (both transposed layout): [128, FT, S] f32r."""
            yT = pool.tile([P, FT, S], F32R, tag=tag)
            for c in range(CH):
                cs = slice(NC_ * c, NC_ * (c + 1))
                for mo in range(FT):
                    pt = ps_t.tile([P, NC_], F32, tag="t")
                    for k in range(kouter):
                        nc.tensor.matmul(pt[:], w_r[:, k, P * mo:P * (mo + 1)],
                                         src[:, k, cs],
                                         start=(k == 0), stop=(k == kouter - 1))
                    nc.vector.tensor_copy(yT[:, mo, cs], pt[:])
            return yT

        def attention(qT, kT, v_sb, n_sk, pool, aT_tag):
            """qT/kT: [128, FT, *] f32r transposed; v_sb: [part, sk_tiles, NH, DH+1].
            Returns aT [128, FT, S] f32r."""
            aT = pool.tile([P, FT, S], F32R, tag=aT_tag)
            sk_tiles = (n_sk + P - 1) // P
            # Head pairs: even head lives on PE rows 0:64, odd on 64:128. Their
            # score matmuls are issued adjacently so the disjoint row-groups run
            # concurrently in the 128x128 array (per-subarray concurrency).
            for hf in range(FT):
                for c in range(CH):
                    cs = slice(NC_ * c, NC_ * (c + 1))
                    ex_tiles = ([], [])
                    for sk in range(sk_tiles):
                        rows = min(P, n_sk - P * sk)
                        for par in range(2):
                            hp = slice(DH * par, DH * par + DH)
                            sc = ps_t.tile([P, NC_], F32, tag="t")
                            nc.tensor.matmul(sc[:rows, :],
                                             kT[hp, hf, P * sk:P * sk + rows],
                                             qT[hp, hf, cs], start=True, stop=True)
                            ex = ex_pool.tile([P, NC_], F32R, tag="ex")
                            nc.scalar.activation(ex[:rows, :], sc[:rows, :], AF.Exp,
                                                 scale=SCALE)
                            ex_tiles[par].append((ex, rows))
                    for par in range(2):
                        h = 2 * hf + par
                        hp = slice(DH * par, DH * par + DH)
                        pv = ps_a.tile([DH + 1, NC_], F32, tag="a")
                        for sk, (ex, rows) in enumerate(ex_tiles[par]):
                            nc.tensor.matmul(pv[:], v_sb[:rows, sk, h, :],
                                             ex[:rows, :], start=(sk == 0),
                                             stop=(sk == sk_tiles - 1))
                        rc = rowp.tile([1, NC_], F32, tag="rc")
                        nc.vector.reciprocal(rc[:], pv[DH:DH + 1, :])
                        bc = bcp.tile([DH, NC_], F32, tag="bcr")
                        nc.gpsimd.partition_broadcast(bc[:], rc[:])
                        nc.vector.tensor_tensor(aT[hp, hf, cs], pv[0:DH, :], bc[:],
                                                ALU.mult)
            return aT

        def out_proj_residual(w_r, bias_c, aT, src, dst_tag, dst_dtype=F32R):
            """dst = src + W.T @ aT + bias (transposed layout)."""
            dst = resid.tile([P, FT, S], dst_dtype, tag=dst_tag)
            for c in range(CH):
                cs = slice(NC_ * c, NC_ * (c + 1))
                for mo in range(FT):
                    pt = ps_t.tile([P, NC_], F32, tag="t")
                    for k in range(FT):
                        nc.tensor.matmul(pt[:], w_r[:, k, P * mo:P * (mo + 1)],
                                         aT[:, k, cs],
                                         start=(k == 0), stop=(k == FT - 1))
                    nc.vector.scalar_tensor_tensor(
                        dst[:, mo, cs], pt[:], bias_c[:, mo:mo + 1], src[:, mo, cs],
                        op0=ALU.add, op1=ALU.add)
            return dst

        # ================= self-attention =================
        with tc.tile_pool(name="a1", bufs=1) as a1:
            with tc.tile_pool(name="a1w", bufs=1) as a1w:
                h1 = layer_norm(xT, g1c, b1c, a1, "h1")
                wq1r = load_w(a1w, "wq1", FT, "wq1r")
                wk1r = load_w(a1w, "wk1", FT, "wk1r")
                wv1r = load_w(a1w, "wv1", FT, "wv1r")
                qT1 = project_T(wq1r, h1, FT, a1, "qT1")
                kT1 = project_T(wk1r, h1, FT, a1, "kT1")
                v1 = a1.tile([P, ST, NH, DH + 1], F32R, tag="v1")
                nc.vector.tensor_copy(
                    v1[:, :, :, DH:DH + 1],
                    ones_f[:, 0:ST * NH].rearrange("p (a b c) -> p a b c",
                                                   a=ST, b=NH))
                for st in range(ST):
                    pt = ps_t.tile([P, NC_], F32, tag="t")
                    for k in range(FT):
                        nc.tensor.matmul(pt[:], h1[:, k, P * st:P * (st + 1)],
                                         wv1r[:, k, :],
                                         start=(k == 0), stop=(k == FT - 1))
                    nc.vector.tensor_copy(
                        v1[:, st, :, 0:DH],
                        pt[:].rearrange("p (h d) -> p h d", h=NH))
            aT1 = attention(qT1, kT1, v1, S, a1, "aT1")
            with tc.tile_pool(name="a1o", bufs=1) as a1o:
                wo1r = load_w(a1o, "wo1", FT, "wo1r")
                xT1 = out_proj_residual(wo1r, bo1c, aT1, xT, "x")

        # ================= cross-attention =================
        with tc.tile_pool(name="a2", bufs=1) as a2:
            h2 = layer_norm(xT1, g2c, b2c, a2, "h2")
            with tc.tile_pool(name="a2wq", bufs=1) as a2wq:
                wq2r = load_w(a2wq, "wq2", FT, "wq2r")
                qT2 = project_T(wq2r, h2, FT, a2, "qT2")
            kT2 = a2.tile([P, FT, SK2], F32R, tag="kT2")
            with tc.tile_pool(name="a2wk", bufs=1) as a2wk:
                wk2r = load_w(a2wk, "wk2", KE, "wk2r")
                for mo in range(FT):
                    pt = ps_t.tile([P, NC_], F32, tag="t")
                    for k in range(KE):
                        nc.tensor.matmul(pt[:, 0:SK2 + 1],
                                         wk2r[:, k, P * mo:P * (mo + 1)],
                                         encT[:, k, :],
                                         start=(k == 0), stop=(k == KE - 1))
                    nc.vector.tensor_copy(kT2[:, mo, :], pt[:, 0:SK2])
            v2 = a2.tile([SK2, 1, NH, DH + 1], F32R, tag="v2")
            nc.vector.tensor_copy(
                v2[:, :, :, DH:DH + 1],
                ones_f[0:SK2, 0:NH].rearrange("p (a b c) -> p a b c", a=1, b=NH))
            with tc.tile_pool(name="a2wv", bufs=1) as a2wv:
                wv2r = load_w(a2wv, "wv2", KE, "wv2r")
                pt = ps_t.tile([P, NC_], F32, tag="t")
                for k in range(KE):
                    nc.tensor.matmul(pt[0:SK2 + 1, :], encT[:, k, :], wv2r[:, k, :],
                                     start=(k == 0), stop=(k == KE - 1))
                nc.vector.tensor_copy(
                    v2[:, 0, :, 0:DH],
                    pt[0:SK2, :].rearrange("p (h d) -> p h d", h=NH))
            aT2 = attention(qT2, kT2, v2, SK2, a2, "aT2")
            with tc.tile_pool(name="a2o", bufs=1) as a2o:
                wo2r = load_w(a2o, "wo2", FT, "wo2r")
                xT2 = out_proj_residual(wo2r, bo2c, aT2, xT1, "x")

        # ================= GEGLU feed-forward =================
        with tc.tile_pool(name="ffp", bufs=1) as ffp, \
             tc.tile_pool(name="wgp", bufs=4) as wgp, \
             tc.tile_pool(name="fft_p", bufs=4) as fft_p:
            h3 = layer_norm(xT2, g3c, b3c, ffp, "h3")
            wfr = load_w(ffp, "wf", NI, "wfr")
            wg_r = w_in["wg"].rearrange("(ko ki) n -> ki ko n", ki=P)
            xT3 = resid.tile([P, FT, S], F32, tag="x")
            for c in range(CH):
                cs = slice(NC_ * c, NC_ * (c + 1))
                wf_ps = [ps_a.tile([P, NC_], F32, tag="a", name=f"wf_ps{c}_{m}")
                         for m in range(FT)]
                for i in range(NI):
                    wgu = stage.tile([P, FT, P], F32, tag="wgst")
                    nc.sync.dma_start(wgu[:], wg_r[:, :, P * i:P * (i + 1)])
                    wgur = wgp.tile([P, FT, P], F32R, tag="wgur")
                    nc.gpsimd.tensor_copy(wgur[:], wgu[:])
                    wgg = stage.tile([P, FT, P], F32, tag="wgst")
                    nc.sync.dma_start(wgg[:], wg_r[:, :, FF + P * i:FF + P * (i + 1)])
                    wggr = wgp.tile([P, FT, P], F32R, tag="wggr")
                    nc.gpsimd.tensor_copy(wggr[:], wgg[:])
                    pu = ps_t.tile([P, NC_], F32, tag="t")
                    for k in range(FT):
                        nc.tensor.matmul(pu[:], wgur[:, k, :], h3[:, k, cs],
                                         start=(k == 0), stop=(k == FT - 1))
                    pg = ps_t.tile([P, NC_], F32, tag="t")
                    for k in range(FT):
                        nc.tensor.matmul(pg[:], wggr[:, k, :], h3[:, k, cs],
                                         start=(k == 0), stop=(k == FT - 1))
                    gel = tmp.tile([P, NC_], F32, tag="lt")
                    nc.scalar.activation(gel[:], pg[:], AF.Gelu,
                                         bias=bgc[:, NI + i:NI + i + 1])
                    fft = fft_p.tile([P, NC_], F32R, tag="fft")
                    nc.vector.scalar_tensor_tensor(
                        fft[:], pu[:], bgc[:, i:i + 1], gel[:],
                        op0=ALU.add, op1=ALU.mult)
                    for m in range(FT):
                        nc.tensor.matmul(wf_ps[m][:], wfr[:, i, P * m:P * (m + 1)],
                                         fft[:], start=(i == 0), stop=(i == NI - 1),
                                         skip_group_check=True)
                for m in range(FT):
                    nc.vector.scalar_tensor_tensor(
                        xT3[:, m, cs], wf_ps[m][:], bfc[:, m:m + 1], xT2[:, m, cs],
                        op0=ALU.add, op1=ALU.add)

        # ================= transpose back & store =================
        for st in range(ST):
            ot = stage.tile([P, D], F32, tag="x_raw")
            pt = ps_t.tile([P, NC_], F32, tag="t")
            for ft in range(FT):
                nc.tensor.transpose(pt[:, P * ft:P * (ft + 1)],
                                    xT3[:, ft, P * st:P * (st + 1)], ident[:])
            nc.vector.tensor_copy(ot[:], pt[:])
            nc.sync.dma_start(out[P * st:P * (st + 1), :], ot[:])

    return nc


_CACHED = {}


def _get_nc():
    if "nc" not in _CACHED:
        nc = bacc.Bacc("TRN2", target_bir_lowering=False, debug=False, num_devices=B)
        build(nc)
        nc.finalize()
        _CACHED["nc"] = nc
    return _CACHED["nc"]


def kernel(**inputs) -> np.ndarray:
    nc = _get_nc()
    x = np.ascontiguousarray(np.asarray(inputs["x"]), dtype=np.float32)
    enc = np.ascontiguousarray(np.asarray(inputs["enc"]), dtype=np.float32)
    shared = {k: np.ascontiguousarray(np.asarray(v), dtype=np.float32)
              for k, v in inputs.items() if k not in ("x", "enc")}
    in_maps = [dict(shared, x=x[i], enc=enc[i]) for i in range(B)]
    res = run_bass_kernel_spmd(nc, in_maps, core_ids=list(range(B)))
    return np.stack([res.results[i]["out"] for i in range(B)], axis=0)


if __name__ == "__main__":
    rng = np.random.default_rng(0)
    demo = {
        "x": rng.standard_normal((B, S, D)).astype(np.float32),
        "enc": rng.standard_normal((B, SK2, DE)).astype(np.float32),
        "wq1": rng.standard_normal((D, D)).astype(np.float32) * 0.02,
    }
    print("module import ok")

